# revision 13
# baseline (speedup 1.0000x reference)
"""Trainium2 Bass kernel: Tacotron-style location-sensitive attention step.

Sharding strategy (8 NeuronCores, SPMD):
  - Attention / conv / softmax / context: pure batch parallelism
    (B=128 -> 16 examples per core; enc_seq, proc_mem, attention weights,
    mask sharded on the batch dim host-side).
  - LSTM cell: H-sharded. Core j computes h.T rows [128j, 128j+128) for the
    FULL batch using only 1/8 of W_ih/W_hh (3.5 MB instead of 28 MB of
    replicated weight traffic), then a small AllGather of h.T.
    qry2 = h @ W_q.T + const is computed for the full batch and each core
    selects its 16 rows with a one-hot matmul (bsel input) so the SPMD
    graph stays core-uniform.

Compute dtypes: bf16 operands for all large matmuls / elementwise (well
inside the 2e-2 rel-err budget), f32 for PSUM, softmax and reductions.

kernel(**inputs) takes FULL numpy inputs (as produced by setup_inputs())
and returns the FULL [128, 512] float32 context.
"""

import sys

sys.path.insert(0, "/opt/trn_rl_repo")

import numpy as np

import concourse.bass as bass
import concourse.mybir as mybir
from concourse import bacc
from concourse.bass_utils import run_bass_kernel_spmd
from concourse.masks import make_identity
from concourse.tile import TileContext

F32 = mybir.dt.float32
BF16 = mybir.dt.bfloat16
AF = mybir.ActivationFunctionType

B, S, E, P, H, A, F, KW = 128, 1024, 512, 256, 1024, 128, 32, 31
NCORES = 8
BL = B // NCORES        # 16 examples per core
HL = H // NCORES        # 128 h rows per core
PE_DIM = P + E          # 768
NKI = PE_DIM // 128     # 6
NKH = H // 128          # 8
NC_S = S // 128         # 8 s-chunks
PADW = KW // 2          # 15
CONVROW = PADW + S + 17  # 1056 padded per-channel staging row
TAPS = 2 * KW           # 62
ENC_T = 8               # s-chunks per enc DMA tile (whole example)
ENC_BUFS = 6


def build():
    nc = bacc.Bacc("TRN2", target_bir_lowering=False, debug=False,
                   num_devices=NCORES)

    dp = nc.declare_dram_parameter
    prenet = dp("prenet", [B, P], F32, isOutput=False)
    prev_ctx = dp("prev_ctx", [B, E], F32, isOutput=False)
    att_h = dp("att_h", [B, H], F32, isOutput=False)
    att_c_sh = dp("att_c_sh", [B, HL], F32, isOutput=False)
    w_ih_sh = dp("w_ih_sh", [4, HL, PE_DIM], F32, isOutput=False)
    w_hh_sh = dp("w_hh_sh", [4, HL, H], F32, isOutput=False)
    b_ih_sh = dp("b_ih_sh", [4, HL], F32, isOutput=False)
    b_hh_sh = dp("b_hh_sh", [4, HL], F32, isOutput=False)
    prev_w = dp("prev_w", [BL, S], F32, isOutput=False)
    cum_w = dp("cum_w", [BL, S], F32, isOutput=False)
    mask = dp("mask", [BL, S], F32, isOutput=False)
    enc = dp("enc", [BL, S, E], F32, isOutput=False)
    proc = dp("proc", [BL, S, A], F32, isOutput=False)
    conv_w = dp("conv_w", [F, 2, KW], F32, isOutput=False)
    conv_b = dp("conv_b", [F, 1], F32, isOutput=False)
    w_loc = dp("w_loc", [A, F], F32, isOutput=False)
    b_loc = dp("b_loc", [1, A], F32, isOutput=False)
    w_q = dp("w_q", [A, H], F32, isOutput=False)
    b_q = dp("b_q", [1, A], F32, isOutput=False)
    w_out = dp("w_out", [1, A], F32, isOutput=False)
    bsel = dp("bsel", [B, BL], F32, isOutput=False)
    out = dp("out", [BL, E], F32, isOutput=True)

    with TileContext(nc) as tc:
        with (
            tc.tile_pool(name="const", bufs=1) as cpool,
            tc.tile_pool(name="work", bufs=2) as wpool,
            tc.tile_pool(name="conv", bufs=16) as convpool,
            tc.tile_pool(name="proc", bufs=4) as ppool,
            tc.tile_pool(name="vbig", bufs=2) as vpool,
            tc.tile_pool(name="enc", bufs=ENC_BUFS) as epool,
            tc.tile_pool(name="psA", bufs=2, space="PSUM") as psA,
            tc.tile_pool(name="psV", bufs=2, space="PSUM") as psV,
            tc.tile_pool(name="psX", bufs=2, space="PSUM") as psX,
            tc.tile_pool(name="dram", bufs=1, space="DRAM") as dpool,
        ):
            def mm_ps(shape):
                t = psA.tile([128, 512], F32, tag="mm")
                return t[: shape[0], : shape[1]]

            # ------------- constants / small preprocessing -------------
            ident = cpool.tile([128, 128], F32)
            make_identity(nc, ident[:])
            ones_row = cpool.tile([1, 128], F32)
            nc.vector.memset(ones_row[:], 1.0)

            def pe_t(dst, src_ap, rows):
                """dst = src_ap([rows, cols]).T via TensorE (+ACT copy/cast)."""
                ps = mm_ps((dst.shape[0], rows))
                nc.tensor.transpose(ps, src_ap, ident[:rows, :rows])
                nc.scalar.copy(dst, ps)

            # conv weights -> w2 [62, 32] (taps on partitions, c-major)
            cw_nat = cpool.tile([F, TAPS], F32)
            nc.sync.dma_start(cw_nat[:], conv_w.rearrange("f c k -> f (c k)"))
            w2 = cpool.tile([TAPS, F], BF16)
            pe_t(w2[:], cw_nat[:], F)

            # W_loc.T [32, 128]
            wl_nat = cpool.tile([A, F], F32)
            nc.sync.dma_start(wl_nat[:], w_loc[:])
            wlocT = cpool.tile([F, A], F32)
            pe_t(wlocT[:], wl_nat[:], A)

            # const_row [1, A] = conv_b @ W_loc.T + b_loc + b_q
            cb_col = cpool.tile([F, 1], F32)
            nc.sync.dma_start(cb_col[:], conv_b[:])
            bl_row = cpool.tile([1, A], F32)
            nc.sync.dma_start(bl_row[:], b_loc[:])
            bq_row = cpool.tile([1, A], F32)
            nc.sync.dma_start(bq_row[:], b_q[:])
            ps = mm_ps((1, A))
            nc.tensor.matmul(ps, cb_col[:], wlocT[:], start=True, stop=True)
            const_row = cpool.tile([1, A], F32)
            nc.vector.tensor_add(const_row[:], ps, bl_row[:])
            nc.vector.tensor_add(const_row[:], const_row[:], bq_row[:])

            # W_out replicated across all 128 partitions, tiled 8x along free
            wo_row = cpool.tile([1, A], F32)
            nc.sync.dma_start(wo_row[:], w_out[:])
            ps = mm_ps((128, A))
            nc.tensor.matmul(ps, ones_row[:], wo_row[:], start=True, stop=True)
            wo_rep8 = cpool.tile([128, NC_S, A], BF16)
            for c in range(NC_S):
                nc.scalar.copy(wo_rep8[:, c, :], ps)

            # LSTM bias columns [HL, 4]
            bi_nat = cpool.tile([4, HL], F32)
            nc.sync.dma_start(bi_nat[:], b_ih_sh[:])
            bh_nat = cpool.tile([4, HL], F32)
            nc.sync.dma_start(bh_nat[:], b_hh_sh[:])
            nc.vector.tensor_add(bi_nat[:], bi_nat[:], bh_nat[:])
            bias_sb = cpool.tile([HL, 4], F32)
            pe_t(bias_sb[:], bi_nat[:], 4)

            sel_sb = cpool.tile([B, BL], F32)
            nc.sync.dma_start(sel_sb[:], bsel[:])
            mask_sb = cpool.tile([BL, S], F32)
            nc.sync.dma_start(mask_sb[:], mask[:])

            # padded conv input rows staged to DRAM:
            # row layout per (b, c): [15 zeros | 1024 data | 17 zeros]
            stage = cpool.tile([BL, 2 * CONVROW], BF16)
            nc.vector.memset(stage[:], 0.0)
            nc.gpsimd.dma_start(stage[:, PADW:PADW + S], cum_w[:])
            nc.gpsimd.dma_start(stage[:, CONVROW + PADW:CONVROW + PADW + S],
                                prev_w[:])
            pad_dram = dpool.tile([BL, 2 * CONVROW], BF16)
            nc.sync.dma_start(pad_dram[:], stage[:])

            # ------------- LSTM operand transposes (bf16) -------------
            NK = NKI + NKH  # 14
            wT = cpool.tile([128, 4, NK, HL], BF16)
            for g in range(4):
                wi_nat = wpool.tile([HL, PE_DIM], F32, tag="wload")
                nc.sync.dma_start(wi_nat[:], w_ih_sh[g])
                for k in range(NKI):
                    pe_t(wT[:, g, k, :], wi_nat[:, k * 128:(k + 1) * 128], HL)
                wh_nat = wpool.tile([HL, H], F32, tag="wload2")
                nc.sync.dma_start(wh_nat[:], w_hh_sh[g])
                for k in range(NKH):
                    pe_t(wT[:, g, NKI + k, :],
                         wh_nat[:, k * 128:(k + 1) * 128], HL)

            inpT = cpool.tile([128, NKI, B], BF16)
            pn_nat = cpool.tile([B, P], F32)
            nc.sync.dma_start(pn_nat[:], prenet[:])
            pc_nat = cpool.tile([B, E], F32)
            nc.sync.dma_start(pc_nat[:], prev_ctx[:])
            for k in range(2):
                pe_t(inpT[:, k, :], pn_nat[:, k * 128:(k + 1) * 128], B)
            for k in range(4):
                pe_t(inpT[:, 2 + k, :], pc_nat[:, k * 128:(k + 1) * 128], B)

            ahT = cpool.tile([128, NKH, B], BF16)
            ah_nat = cpool.tile([B, H], F32)
            nc.sync.dma_start(ah_nat[:], att_h[:])
            for k in range(NKH):
                pe_t(ahT[:, k, :], ah_nat[:, k * 128:(k + 1) * 128], B)

            ac_nat = cpool.tile([B, HL], F32)
            nc.sync.dma_start(ac_nat[:], att_c_sh[:])
            acT = cpool.tile([HL, B], BF16)
            pe_t(acT[:], ac_nat[:], B)

            wq_nat = cpool.tile([A, H], F32)
            nc.sync.dma_start(wq_nat[:], w_q[:])
            wqT = cpool.tile([128, NKH, A], BF16)
            for k in range(NKH):
                pe_t(wqT[:, k, :], wq_nat[:, k * 128:(k + 1) * 128], A)

            # ------------- LSTM gates (H-shard, full batch) -------------
            gate_sb = []
            for g in range(4):
                ps = mm_ps((HL, B))
                for k in range(NKI):
                    nc.tensor.matmul(ps, wT[:, g, k, :], inpT[:, k, :],
                                     start=(k == 0), stop=False)
                for k in range(NKH):
                    nc.tensor.matmul(ps, wT[:, g, NKI + k, :], ahT[:, k, :],
                                     start=False, stop=(k == NKH - 1))
                sb = cpool.tile([HL, B], BF16, tag=f"gate{g}")
                fn = AF.Tanh if g == 2 else AF.Sigmoid
                nc.scalar.activation(sb[:], ps, fn, bias=bias_sb[:, g:g + 1])
                gate_sb.append(sb)

            cT = cpool.tile([HL, B], BF16)
            nc.vector.tensor_mul(cT[:], gate_sb[1][:], acT[:])
            tg = cpool.tile([HL, B], BF16)
            nc.vector.tensor_mul(tg[:], gate_sb[0][:], gate_sb[2][:])
            nc.vector.tensor_add(cT[:], cT[:], tg[:])
            nc.scalar.activation(tg[:], cT[:], AF.Tanh)
            hT_sh = cpool.tile([HL, B], BF16)
            nc.vector.tensor_mul(hT_sh[:], gate_sb[3][:], tg[:])

            # early proc prefetch (sync queue, before h_in's gate-wait)
            proc_tiles = []
            for b in range(4):
                pt = ppool.tile([128, NC_S, A], F32, tag="proc")
                nc.sync.dma_start(
                    pt[:], proc[b].rearrange("(c p) a -> p c a", p=128))
                proc_tiles.append(pt)

            h_in = dpool.tile([HL, B], BF16)
            nc.sync.dma_start(h_in[:], hT_sh[:])

            # early enc prefetch: exactly ENC_BUFS-1 tiles so the gpsimd
            # queue never blocks on a slot before the collective issues
            enc_tiles = []
            for b in range(ENC_BUFS - 1):
                et = epool.tile([128, ENC_T, E], BF16, tag="enc")
                nc.gpsimd.dma_start(
                    et[:], enc[b].rearrange("(t p) e -> p t e", p=128))
                enc_tiles.append(et)

            # ------------- AllGather h.T -------------
            h_gat = dpool.tile([NCORES, HL, B], BF16)
            nc.gpsimd.collective_compute(
                "AllGather",
                mybir.AluOpType.bypass,
                replica_groups=[list(range(NCORES))],
                ins=[h_in[:].opt()],
                outs=[h_gat[:].opt()],
            )

            # ------------- location conv (overlaps the collective) -------
            conv_tiles = []
            for b in range(BL):
                xpadT = wpool.tile([TAPS, S], BF16, tag="xpad")
                for c in range(2):
                    base = pad_dram[b, c * CONVROW:c * CONVROW + 1]
                    src = bass.AP(
                        tensor=base.tensor,
                        offset=base.offset,
                        ap=[[1, KW], [1, S]],
                    )
                    nc.sync.dma_start(xpadT[c * KW:(c + 1) * KW, :], src)
                conv_sb = convpool.tile([F + 1, S], BF16, tag="conv")
                for h2 in range(2):
                    ps = mm_ps((F, 512))
                    nc.tensor.matmul(ps, w2[:],
                                     xpadT[:, h2 * 512:(h2 + 1) * 512],
                                     start=True, stop=True)
                    nc.scalar.copy(conv_sb[:F, h2 * 512:(h2 + 1) * 512], ps)
                nc.vector.memset(conv_sb[F:F + 1, :], 1.0)
                conv_tiles.append(conv_sb)

            hfull = cpool.tile([128, NKH, B], BF16)
            nc.sync.dma_start(hfull[:], h_gat[:].rearrange("c p b -> p c b"))

            # ------------- qry2 (full batch) + batch selection -------------
            ps_q = mm_ps((B, A))
            for k in range(NKH):
                nc.tensor.matmul(ps_q, hfull[:, k, :], wqT[:, k, :],
                                 start=(k == 0), stop=False)
            nc.tensor.matmul(ps_q, ones_row[:], const_row[:],
                             start=False, stop=True)
            qry2_all = cpool.tile([B, A], F32)
            nc.scalar.copy(qry2_all[:], ps_q)
            ps_q2 = mm_ps((BL, A))
            nc.tensor.matmul(ps_q2, sel_sb[:], qry2_all[:],
                             start=True, stop=True)
            qry2 = cpool.tile([BL, A], F32)
            nc.scalar.copy(qry2[:], ps_q2)

            # rhs_all[:, b, :] = [W_loc.T ; qry2[b]]  (K=33 fused loc+qry mm)
            rhs_all = cpool.tile([F + 1, BL, A], BF16)
            for b in range(BL):
                nc.scalar.copy(rhs_all[:F, b, :], wlocT[:])
            qdram = dpool.tile([BL, A], F32)
            nc.sync.dma_start(qdram[:], qry2[:])
            qsrc = bass.AP(
                tensor=qdram[:].tensor,
                offset=qdram[:].offset,
                ap=[[BL * A, 1], [A, BL], [1, A]],
            )
            nc.gpsimd.dma_start(rhs_all[F:F + 1, :, :], qsrc)

            # remaining streaming loads (queues can stall safely now)
            for b in range(4, BL):
                pt = ppool.tile([128, NC_S, A], F32, tag="proc")
                nc.sync.dma_start(
                    pt[:], proc[b].rearrange("(c p) a -> p c a", p=128))
                proc_tiles.append(pt)
            for b in range(ENC_BUFS - 1, BL):
                et = epool.tile([128, ENC_T, E], BF16, tag="enc")
                nc.gpsimd.dma_start(
                    et[:], enc[b].rearrange("(t p) e -> p t e", p=128))
                enc_tiles.append(et)

            # ------------- scores per example -------------
            scoresT = cpool.tile([128, NC_S, BL], F32)
            for b in range(BL):
                conv_sb = conv_tiles[b]
                ps_v = psV.tile([128, NC_S * A], F32, tag="v")
                for c in range(NC_S):
                    nc.tensor.matmul(
                        ps_v[:, c * A:(c + 1) * A],
                        conv_sb[:, c * 128:(c + 1) * 128],
                        rhs_all[:, b, :],
                        start=True, stop=True)
                v_sb = vpool.tile([128, NC_S, A], BF16, tag="v_sb")
                nc.vector.tensor_add(
                    v_sb[:],
                    ps_v[:].rearrange("p (c a) -> p c a", c=NC_S),
                    proc_tiles[b][:])
                nc.scalar.activation(v_sb[:], v_sb[:], AF.Tanh)
                nc.vector.tensor_mul(v_sb[:], v_sb[:], wo_rep8[:])
                nc.vector.reduce_sum(scoresT[:, :, b], v_sb[:],
                                     axis=mybir.AxisListType.X)

            # ------------- softmax over S in [b, s] layout -------------
            sc = cpool.tile([BL, S], F32)
            for c in range(NC_S):
                pe_t(sc[:, c * 128:(c + 1) * 128], scoresT[:, c, :], 128)
            nc.vector.tensor_add(sc[:], sc[:], mask_sb[:])
            mx = cpool.tile([BL, 1], F32)
            nc.vector.reduce_max(mx[:], sc[:], axis=mybir.AxisListType.X)
            nc.vector.tensor_scalar_mul(mx[:], mx[:], -1.0)
            sums = cpool.tile([BL, 1], F32)
            nc.scalar.activation(sc[:], sc[:], AF.Exp, bias=mx[:],
                                 accum_out=sums[:])
            rs = cpool.tile([BL, 1], F32)
            nc.vector.reciprocal(rs[:], sums[:])
            nc.vector.tensor_scalar_mul(sc[:], sc[:], rs[:])

            wTt = cpool.tile([128, NC_S, BL], BF16)
            for c in range(NC_S):
                pe_t(wTt[:, c, :], sc[:, c * 128:(c + 1) * 128], BL)

            # ------------- context = weights @ enc_seq -------------
            for b in range(BL):
                ps_x = psX.tile([1, E], F32, tag="ctx")
                for c in range(NC_S):
                    nc.tensor.matmul(ps_x, wTt[:, c, b:b + 1],
                                     enc_tiles[b][:, c, :],
                                     start=(c == 0), stop=(c == NC_S - 1))
                ctx_row = wpool.tile([1, E], F32, tag="ctxrow")
                nc.scalar.copy(ctx_row[:], ps_x)
                nc.sync.dma_start(out[b:b + 1, :], ctx_row[:])

    nc.compile()
    return nc


_NC_CACHE = None


def _get_nc():
    global _NC_CACHE
    if _NC_CACHE is None:
        _NC_CACHE = build()
    return _NC_CACHE


def shard_inputs(prenet, prev_context, att_h, att_c, prev_weights, cum_weights,
                 enc_seq, proc_mem, mask, W_ih, W_hh, b_ih, b_hh, conv_w,
                 conv_b, W_loc, b_loc, W_q, b_q, W_out, **_unused):
    f = np.ascontiguousarray
    w_ih4 = np.asarray(W_ih, np.float32).reshape(4, H, PE_DIM)
    w_hh4 = np.asarray(W_hh, np.float32).reshape(4, H, H)
    b_ih4 = np.asarray(b_ih, np.float32).reshape(4, H)
    b_hh4 = np.asarray(b_hh, np.float32).reshape(4, H)
    in_maps = []
    for j in range(NCORES):
        bj = slice(BL * j, BL * (j + 1))
        hj = slice(HL * j, HL * (j + 1))
        sel = np.zeros((B, BL), np.float32)
        sel[BL * j:BL * (j + 1), :] = np.eye(BL, dtype=np.float32)
        in_maps.append({
            "prenet": f(np.asarray(prenet, np.float32)),
            "prev_ctx": f(np.asarray(prev_context, np.float32)),
            "att_h": f(np.asarray(att_h, np.float32)),
            "att_c_sh": f(np.asarray(att_c, np.float32)[:, hj]),
            "w_ih_sh": f(w_ih4[:, hj]),
            "w_hh_sh": f(w_hh4[:, hj]),
            "b_ih_sh": f(b_ih4[:, hj]),
            "b_hh_sh": f(b_hh4[:, hj]),
            "prev_w": f(np.asarray(prev_weights, np.float32)[bj]),
            "cum_w": f(np.asarray(cum_weights, np.float32)[bj]),
            "mask": f(np.asarray(mask, np.float32)[bj]),
            "enc": f(np.asarray(enc_seq, np.float32)[bj]),
            "proc": f(np.asarray(proc_mem, np.float32)[bj]),
            "conv_w": f(np.asarray(conv_w, np.float32)),
            "conv_b": f(np.asarray(conv_b, np.float32).reshape(F, 1)),
            "w_loc": f(np.asarray(W_loc, np.float32)),
            "b_loc": f(np.asarray(b_loc, np.float32).reshape(1, A)),
            "w_q": f(np.asarray(W_q, np.float32)),
            "b_q": f(np.asarray(b_q, np.float32).reshape(1, A)),
            "w_out": f(np.asarray(W_out, np.float32).reshape(1, A)),
            "bsel": sel,
        })
    return in_maps


def kernel(**inputs):
    nc = _get_nc()
    in_maps = shard_inputs(**inputs)
    res = run_bass_kernel_spmd(nc, in_maps, core_ids=list(range(NCORES)))
    return np.concatenate([res.results[j]["out"] for j in range(NCORES)],
                          axis=0)


if __name__ == "__main__":
    rng = np.random.default_rng(0)
    print("building...")
    _get_nc()
    print("built ok")
